# revision 40
# baseline (speedup 1.0000x reference)
"""Trainium2 Bass kernel for nn_Attention (Quad2-normalized multi-head attention).

Problem: B=8, N=1024, C=768, H=12 heads, head_dim=64.
  qkv = x @ qkv_w.T ; per head: s = q @ k.T ; t = (s/8 + 5)^2
  attn = t / rowsum(t) ; out = attn @ v ; y = out @ proj_w.T + proj_b
Sharding: data-parallel over batch B - one batch element per NeuronCore.

Layout (feature-major so PE contraction lands on partitions): xt = x[b].T
[768,1024] bf16; packed projections qkt = Wqk @ x.T in [feat, n]; V =
x @ Wv.T in [n, feat], ones-AUGMENTED (65 cols/head) so the AV matmul
(M=65) emits the row-sum r on psum row 64 for free; t = (s+40)^2 (the 64x
scale vs the reference cancels in the normalization).

HW-measured PE model this schedule is built around (slope-benchmarked on
the real device; the CoreSim cost model does NOT reflect these):
  - matmul cost ~= F*K/128 cycles at 2.4 GHz for K in (80,128], but K<=80
    falls into a 2 cycles-per-column slow mode (2x cost);
  - every matmul pays a serial ~107-124ns LDWEIGHTS tax (M columns /
    1.2GHz, never hidden, never deduped by this toolchain);
  - gpsimd (Q7) ops and SWDGE DMA dispatch are far slower than modeled -
    gpsimd is used only for memsets and a few yt DMAs.

Main techniques:
  - Scores contract at K=96 instead of the slow K=64: zero-padded offset-0
    operand copies qaP=[qA(64),0(32)], qbP=[qB(64),0(32)], kkb=[kB(64),
    junk(32)] are derived from the packed projection tiles by cheap SBUF
    DMAs + pad memsets (junk rows are cancelled by zero rows on the other
    operand). Head A uses kk[0:96] x qaP[0:96] directly.
  - The 1/r broadcast matmuls are also K=96: a one-hot [96,128] weights
    tile (row 0 / row 32 hot) against a zero-padded [96,N] recip tile
    replaces the K=1 ones-matmul (which ran in the slow mode).
  - Normalization of pair p is deferred TWO pairs (consumed at pair p+2):
    the serial chain usb-eviction -> r-row DMA (psum partition 64 ->
    rows {0,32} of the gather tile) -> batched DVE reciprocal -> bf16
    copy has a full window to complete and never stalls the PE. (Custom
    DVE ISA ops mis-address at partition offsets > 0, so the reciprocal
    always runs on an offset-0 AP.)
  - Both heads' AV matmuls interleave into the score window (A lags 2
    m-tiles, B lags 3); square evictions balanced ACT/DVE via DVE_EVICT.
  - PSUM (8 banks): "st" pool 2x[128,1024] for scores/projections/V plus
    "u" pool shared round-robin by ups0/ups1 (AV accumulators) and the
    bc broadcast tiles, in the order bc0(p-2), bc1(p-2), ups0(p), ups1(p).
  - proj splits head/mid/last and interleaves with the two tail
    normalizes so the PE stays fed to the end.
"""

import contextlib

import numpy as np

TRACE = False
TRACE_KWARGS = {}
LAST_RESULT = None
SIM_SAFE = False  # unused in v2 (no partially-initialized tiles remain)

B, N, C = 8, 1024, 768
H, HD = 12, 64
NT = N // 128      # 8 n/m tiles
CT = C // 128      # 6 feature tiles
SC = 512           # psum-bank chunk of the free dim
T_BUFS = 12
USB_BUFS = 6
# (mt, hoff) square evictions routed to DVE instead of ACT
DVE_EVICT = ((5, 0), (5, 64))

# v3/v4 experiment flags
NOLOAD = True      # ldweights=False on the 2nd matmul of same-lhsT pairs
SPLIT_Q = True     # wv inputs on the ACT HWDGE queue (SP otherwise)
OUT_BF16 = True    # yt in bf16 (host upcasts); halves output DMA
TAIL_SPLIT = True  # column-halved normalize chain for the last pair
START_SPLIT = False  # xt half-DMAs measured ~neutral-to-worse on HW
TAIL_COPY_DVE = True  # alternate final ysb copies between ACT and DVE
BCAST_NORM = False  # normalize via DVE partition-broadcast mul (no bc mms)
BC_MERGE = True    # one M=128 bc matmul for both heads (vs two M=64)
# Fold the +40 bias into the scores matmul (bias row hiding in the K=96
# zero-pad region at row 95: q side 1.0, k side 40.0) so a DVE eviction is
# ONE tensor_mul(ps,ps) instead of add+mul; then rebalance evictions
# ACT<->DVE (HW ACT eviction ~1.18us each can't keep up with 2
# matmuls/eviction on its own). K stays 96 - the PE's fast sweet spot
# (ceil(K/32)-quantized cost; 97 would pay the full 128-row rate).
BIAS_FOLD = False  # dead: DVE has no 1-op square (pow + dual-PSUM both fail)
DVE_EVICT_FOLD = ((1, 0), (2, 64), (3, 0), (4, 64), (5, 0), (6, 64))
# Ring map v7: SP carries inputs (+pads) ONLY so the next For_i iteration's
# input reload prefetches instead of queuing behind tail-gated output
# descriptors (FIFO head-of-line); outputs dispatch from ACT right after
# their own copies (always-satisfied deps); small DVE-dependent DMAs
# (r-rows, ottmp) go to the Pool ring.
RING_V7 = True
HOIST_INPUTS = False  # diagnostic: input DMAs outside the For_i loop

_CACHE = {}


def _raw_matmul(nc, mybir, out, lhsT, rhs, start=True, stop=True):
    """nc.tensor.matmul clone emitting InstMatmult(ldweights=False): reuses
    the PE-resident weights loaded by the immediately preceding matmul with
    the same lhsT (bf16 only; pairs must be adjacent in PE program order)."""
    eng = nc.tensor
    keep_dims = {0}
    ifmap_ap = eng.lower_ap(rhs.opt(keep_dims), opt=False)
    weights_ap = eng.lower_ap(lhsT.opt(keep_dims), opt=False,
                              for_matmul_weights=True)
    out_ap = eng.lower_ap(out)
    assert lhsT.base_partition() == rhs.base_partition()

    def _round_up(size):
        for v in (32, 64, 128):
            if v >= size:
                return v
        raise AssertionError(size)

    inst = mybir.InstMatmult(
        name=nc.get_next_instruction_name(),
        replication_resolution=0,
        replication_shift_amnt=0,
        replication_num_rows=0,
        start_tensor_calc=start,
        stop_tensor_calc=stop,
        ins=[ifmap_ap, weights_ap],
        outs=[out_ap],
        perf_mode=None,
        is_transpose=None,
        ifmap_quant_offset=None,
        weights_quant_offset=None,
        bass_skip_group_check=False,
        tile_position=(lhsT.base_partition(), out.base_partition()),
        tile_size=(_round_up(rhs.partition_size()),
                   _round_up(out.partition_size())),
        ldweights=False,
    )
    return eng.add_instruction(inst)


def _ensure_path():
    import sys
    for p in ("/opt/trn_rl_repo", "/root/.axon_site/_ro/trn_rl_repo"):
        if p not in sys.path:
            sys.path.insert(0, p)


def _build_nc(loop_n=None):
    import concourse.bacc as bacc
    import concourse.mybir as mybir
    import concourse.tile as tile

    f32 = mybir.dt.float32
    bf16 = mybir.dt.bfloat16
    AF = mybir.ActivationFunctionType

    nc = bacc.Bacc("TRN2", target_bir_lowering=False)
    ot_dt = bf16 if OUT_BF16 else f32
    xt_d = nc.dram_tensor("xt", [C, N], bf16, kind="ExternalInput")
    wqk_d = nc.dram_tensor("wqk", [C, 2 * C], bf16, kind="ExternalInput")
    wv_d = nc.dram_tensor("wv", [C, C], bf16, kind="ExternalInput")
    wp_d = nc.dram_tensor("wp", [C, C], bf16, kind="ExternalInput")
    yt_d = nc.dram_tensor("yt", [C, N], ot_dt, kind="ExternalOutput")

    with tile.TileContext(nc) as tc:
        with (
            tc.tile_pool(name="pw", bufs=1) as pw,
            tc.tile_pool(name="pq", bufs=1) as pq,
            tc.tile_pool(name="pt", bufs=T_BUFS) as pt,
            tc.tile_pool(name="pu", bufs=2) as pu,
            tc.tile_pool(name="psa", bufs=2, space="PSUM") as psa,
            tc.tile_pool(name="psu", bufs=2, space="PSUM") as psu,
        ):
            mm = nc.tensor.matmul

            def mm2(out, lhsT, rhs, start=True, stop=True, second=False):
                # second=True: this matmul immediately follows one with the
                # SAME lhsT on the PE queue - skip the redundant LDWEIGHTS
                if second and NOLOAD:
                    return _raw_matmul(nc, mybir, out, lhsT, rhs,
                                       start=start, stop=stop)
                return mm(out, lhsT, rhs, start=start, stop=stop)

            # second HWDGE ring (ACT engine) for wv/wp inputs + pad DMAs
            dma2 = nc.scalar if SPLIT_Q else nc.sync

            bias40 = pw.tile([128, 1], f32, tag="bias40", name="bias40")
            ones_bc = pw.tile([96, 128], bf16, tag="ones_bc", name="ones_bc")
            nc.gpsimd.memset(bias40[:], 0.0 if BIAS_FOLD else 40.0)
            nc.gpsimd.memset(ones_bc[:], 0.0)
            nc.gpsimd.memset(ones_bc[0:1, 0:64], 1.0)
            nc.gpsimd.memset(ones_bc[32:33, 64:128], 1.0)
            SK = 96  # score contraction depth (the PE's fast sweet spot)
            dve_evict = set(DVE_EVICT_FOLD if BIAS_FOLD else DVE_EVICT)
            if BIAS_FOLD:
                # constant rows for the folded-bias trick; Pool memsets need
                # 32-aligned partition starts, so single rows at partition
                # 95 are written by tiny SBUF DMAs from these instead
                c1_row = pw.tile([1, N], bf16, tag="c1row", name="c1row")
                c40_row = pw.tile([1, N], bf16, tag="c40row", name="c40row")
                nc.gpsimd.memset(c1_row[:], 1.0)
                nc.gpsimd.memset(c40_row[:], 40.0)

            loop_ctx = tc.For_i(0, loop_n, 1) if loop_n else contextlib.nullcontext()
            loop_stack = contextlib.ExitStack()

            # ---- input DMAs, first-use order; with RING_V7 everything on
            # the SP ring (which carries ONLY inputs+pads so the next
            # iteration's reload prefetches); otherwise wv on the ACT ring
            wv_sb = []
            xt_sb = []
            wqk_sb = []
            wp_sb = []

            def emit_inputs():
                in2 = nc.sync if RING_V7 else dma2
                for k in range(CT):
                    t_ = pw.tile([128, N], bf16, tag=f"xt{k}", name=f"xt{k}")
                    if START_SPLIT:
                        # first halves only; second halves stream in below.
                        # The V phase's early m-tiles touch only columns <
                        # 512, so the PE starts ~2x sooner on a cold ring.
                        nc.sync.dma_start(
                            out=t_[:, 0:SC],
                            in_=xt_d[k * 128:(k + 1) * 128, 0:SC])
                    else:
                        nc.sync.dma_start(
                            out=t_[:], in_=xt_d[k * 128:(k + 1) * 128, :])
                    xt_sb.append(t_)
                    t_ = pw.tile([128, C], bf16, tag=f"wv{k}", name=f"wv{k}")
                    in2.dma_start(out=t_[:],
                                  in_=wv_d[k * 128:(k + 1) * 128, :])
                    wv_sb.append(t_)
                if START_SPLIT:
                    for k in range(CT):
                        nc.sync.dma_start(
                            out=xt_sb[k][:, SC:N],
                            in_=xt_d[k * 128:(k + 1) * 128, SC:N])
                for k in range(CT):
                    t_ = pw.tile([128, 2 * C], bf16, tag=f"wqk{k}",
                                 name=f"wqk{k}")
                    nc.sync.dma_start(out=t_[:],
                                      in_=wqk_d[k * 128:(k + 1) * 128, :])
                    wqk_sb.append(t_)
                for k in range(CT):
                    t_ = pw.tile([128, C], bf16, tag=f"wp{k}", name=f"wp{k}")
                    nc.sync.dma_start(out=t_[:],
                                      in_=wp_d[k * 128:(k + 1) * 128, :])
                    wp_sb.append(t_)

            if HOIST_INPUTS:
                emit_inputs()
                loop_stack.enter_context(loop_ctx)
            else:
                loop_stack.enter_context(loop_ctx)
                emit_inputs()

            # ---- V = x @ Wv.T, [n, feat] layout, ones-augmented (65 cols/head,
            # ones at 65h+64) so AV's M=65 also produces the row-sum r
            vv = []
            for mt in range(NT):
                ps = psa.tile([128, C], f32, tag="st", name=f"vps{mt}")
                for (o, w) in ((0, SC), (SC, C - SC)):
                    for k in range(CT):
                        mm(ps[:, o:o + w],
                           xt_sb[k][:, mt * 128:(mt + 1) * 128],
                           wv_sb[k][:, o:o + w],
                           start=(k == 0), stop=(k == CT - 1))
                v_ = pq.tile([128, H * 65], bf16, tag=f"vv{mt}", name=f"vv{mt}")
                nc.gpsimd.memset(v_[:], 1.0)
                vdst = v_[:, :].rearrange("p (h s) -> p h s", s=65)[:, :, 0:64]
                vsrc = ps[:, 0:C].rearrange("p (h s) -> p h s", s=64)
                nc.vector.tensor_copy(vdst, vsrc)
                vv.append(v_)

            # ---- QK^T = Wqk @ x.T, [feat, n] layout, packed projections
            # (pair p: Q tile = heads 2p,2p+1 stacked; K tile likewise).
            # Scores contract over K=96 because the PE runs a 2x-slower mode
            # for K<=65: each score operand gets a zero-padded offset-0 copy
            # ([data(64), 0(32)]) derived by one SBUF DMA + a memset of the
            # pad rows. qaP/qbP pads zero the junk rows of the kk/kkb lhsT.
            qkt = {}

            def emit_qtile(p, which):
                # which: 0 = packed Q tile, 2 = packed K tile
                col0 = 128 * p + (C if which == 2 else 0)
                ps = psa.tile([128, N], f32, tag="st", name=f"qkps{p}_{which}")
                for c in range(2):
                    for k in range(CT):
                        mm(ps[:, c * SC:(c + 1) * SC],
                           wqk_sb[k][:, col0:col0 + 128],
                           xt_sb[k][:, c * SC:(c + 1) * SC],
                           start=(k == 0), stop=(k == CT - 1))
                q_ = pq.tile([128, N], bf16, tag=f"qkt{which}", bufs=2,
                             name=f"qkt{p}_{which}")
                nc.vector.tensor_copy(q_[:], ps[:])
                qkt[(p, which)] = q_
                if which == 0:
                    for hi, tag in ((0, "qaP"), (1, "qbP")):
                        t_ = pq.tile([SK, N], bf16, tag=tag, bufs=3,
                                     name=f"{tag}{p}")
                        nc.gpsimd.memset(t_[64:96, :], 0.0)
                        if BIAS_FOLD:
                            # bias row hiding in the zero pad: q side 1.0
                            # x k side 40.0 adds +40 inside the matmul
                            nc.sync.dma_start(out=t_[95:96, :],
                                              in_=c1_row[:, :])
                        nc.sync.dma_start(out=t_[0:64, :],
                                          in_=q_[64 * hi:64 * hi + 64, :])
                        qkt[(p, 3 + hi)] = t_
                else:
                    t_ = pq.tile([SK, N], bf16, tag="kkb", bufs=3,
                                 name=f"kkb{p}")
                    nc.gpsimd.memset(t_[64:96, :], 0.0)
                    nc.sync.dma_start(out=t_[0:64, :], in_=q_[64:128, :])
                    if BIAS_FOLD:
                        nc.sync.dma_start(out=t_[95:96, :], in_=c40_row[:, :])
                        # head A reads kk = q_ directly; its row 95 (head
                        # B data, already copied to kkb above) becomes the
                        # 40.0 bias row - overwrite AFTER the kkb DMA read
                        nc.sync.dma_start(out=q_[95:96, :], in_=c40_row[:, :])
                    qkt[(p, 5)] = t_

            for p0 in (0, 1):
                emit_qtile(p0, 0)
                emit_qtile(p0, 2)

            # ---- attention pairs. State carried across pairs:
            # fin[p] = {"usbs", "rgs", "rrs"} after pair p's AV completes;
            # rrs[hi] bf16 [1, N] ready ~one pair later; normalize(p) at p+2.
            ot = [None] * 6
            fin = [None] * 6

            def tail_cols(p):
                # column-halved chain for the last pair: each stage's first
                # half unblocks the next stage ~0.5us earlier
                if TAIL_SPLIT and p == 5:
                    return ((0, SC), (SC, SC))
                return ((0, N),)

            def finish_head(p, hi, ups_hi, usbs, rgs):
                # U+r eviction; r row (psum partition 64) -> partition {0,32}
                # of the pair's gather tile; recip batched in the NEXT window
                usb = pu.tile([65, N], f32, tag="usb", bufs=USB_BUFS,
                              name=f"usb{p}_{hi}")
                if hi == 0:
                    rgs[0] = pu.tile([33, N], f32, tag="rg", bufs=2,
                                     name=f"rg{p}")
                    # rows 1:31 are never DMA-written; the K=96 bcast reads
                    # their reciprocals against zero weights, and 0*Inf/NaN
                    # would poison the accumulation - keep them finite
                    nc.gpsimd.memset(rgs[0][:], 1.0)
                row = hi * 32
                reng = nc.gpsimd if RING_V7 else nc.sync
                for (o, w) in tail_cols(p):
                    nc.vector.tensor_copy(usb[:, o:o + w], ups_hi[:, o:o + w])
                    reng.dma_start(out=rgs[0][row:row + 1, o:o + w],
                                   in_=usb[64:65, o:o + w])
                usbs[hi] = usb

            def emit_recips(p):
                # pair p's batched reciprocal, emitted mid-window of pair p+1
                rg = fin[p]["rgs"][0]
                rrf = pu.tile([33, N], f32, tag="rrf", bufs=2, name=f"rrf{p}")
                if BCAST_NORM:
                    for (o, w) in tail_cols(p):
                        nc.vector.reciprocal_approx_fast(rrf[:, o:o + w],
                                                         rg[:, o:o + w])
                    fin[p]["rrf"] = rrf
                    return
                rr = pu.tile([96, N], bf16, tag="rr", bufs=2, name=f"rr{p}")
                nc.gpsimd.memset(rr[32:64, :], 0.0)
                nc.gpsimd.memset(rr[64:96, :], 0.0)
                for (o, w) in tail_cols(p):
                    nc.vector.reciprocal_approx_fast(rrf[:, o:o + w],
                                                     rg[:, o:o + w])
                    nc.vector.tensor_copy(rr[0:33, o:o + w], rrf[:, o:o + w])
                fin[p]["rrs"][0] = rr
                fin[p]["rrs"][1] = rr

            def alloc_bc(p):
                # slot-order bookkeeping: bc tiles must take "u" slots before
                # the new pair's ups0/ups1 (round-robin per tag)
                if BCAST_NORM:
                    return
                if BC_MERGE:
                    bc = psu.tile([128, N], f32, tag="u", name=f"bc{p}")
                    fin[p]["bcs"] = [bc, bc]
                    return
                fin[p]["bcs"] = [
                    psu.tile([64, N], f32, tag="u", name=f"bc{p}_{hi}")
                    for hi in (0, 1)
                ]

            def emit_norm_head(p, hi):
                # pair p's normalize for one head: 1/r broadcast across the
                # 64 feature partitions (PE ones-bcast, or a DVE stride-0
                # partition-broadcast read), then the DVE mul; odd head
                # lands in ot via SBUF DMA
                usbs = fin[p]["usbs"]
                if BCAST_NORM:
                    row = 32 * hi
                    rrf = fin[p]["rrf"]

                    def rsrc(o, w):
                        return rrf[row:row + 1, o:o + w].partition_broadcast(64)
                elif BC_MERGE:
                    # one M=128 matmul fills 1/rA into rows 0:64 and 1/rB
                    # into rows 64:128 (ones_bc row 0 hot for cols 0:64,
                    # row 32 hot for cols 64:128); emitted with head 0
                    rrs = fin[p]["rrs"]
                    bc = fin[p]["bcs"][hi]
                    if hi == 0:
                        for c in range(2):
                            cs = slice(c * SC, (c + 1) * SC)
                            mm2(bc[:, cs], ones_bc[0:96, 0:128],
                                rrs[0][0:96, cs],
                                start=True, stop=True, second=(c == 1))

                    def rsrc(o, w):
                        return bc[64 * hi:64 * hi + 64, o:o + w]
                else:
                    rrs = fin[p]["rrs"]
                    bc = fin[p]["bcs"][hi]
                    for c in range(2):
                        cs = slice(c * SC, (c + 1) * SC)
                        mm2(bc[:, cs], ones_bc[0:96, 64 * hi:64 * hi + 64],
                            rrs[hi][0:96, cs],
                            start=True, stop=True, second=(c == 1))

                    def rsrc(o, w):
                        return bc[:, o:o + w]
                if hi == 0:
                    o_ = pq.tile([128, N], bf16, tag=f"ot{p}", name=f"ot{p}")
                    ot[p] = o_
                    for (o, w) in tail_cols(p):
                        nc.vector.tensor_mul(o_[0:64, o:o + w],
                                             usbs[0][0:64, o:o + w],
                                             rsrc(o, w))
                else:
                    o_ = ot[p]
                    ottmp = pu.tile([64, N], bf16, tag="ottmp", bufs=2,
                                    name=f"ottmp{p}")
                    oeng = nc.gpsimd if RING_V7 else nc.sync
                    for (o, w) in tail_cols(p):
                        nc.vector.tensor_mul(ottmp[:, o:o + w],
                                             usbs[1][0:64, o:o + w],
                                             rsrc(o, w))
                        oeng.dma_start(out=o_[64:128, o:o + w],
                                       in_=ottmp[:, o:o + w])

            def emit_normalize(p):
                alloc_bc(p)
                emit_norm_head(p, 0)
                emit_norm_head(p, 1)

            for p in range(6):
                kk = qkt[(p, 2)]
                qaP = qkt[(p, 3)]
                qbP = qkt[(p, 4)]
                kkb = qkt[(p, 5)]
                usbs = {}
                rgs = {}
                ts = {}
                ups = {}
                fin[p] = {"usbs": usbs, "rgs": rgs, "rrs": {}}

                def emit_av(hi, k, c):
                    cs = slice(c * SC, (c + 1) * SC)
                    h = 2 * p + hi
                    mm2(ups[hi][:, cs],
                        vv[k][:, h * 65:h * 65 + 65],
                        ts[(k, hi * 64)][:, cs],
                        start=(k == 0), stop=(k == NT - 1),
                        second=(c == 1))

                for mt in range(NT):
                    if mt == 0 and p >= 2:
                        alloc_bc(p - 2)
                    # scores at K=96/97 (K<=80 runs a 2x-slower PE mode):
                    # head A: kk[0:SK]  x qaP[0:SK]  (qaP rows 64:96 zero)
                    # head B: kkb[0:SK] x qbP[0:SK]  (kkb rows 64:96 zero)
                    # with BIAS_FOLD, row 96 is the +40 bias row
                    m0 = mt * 128
                    for hoff in (0, 64):
                        ps = psa.tile([128, N], f32, tag="st",
                                      name=f"sps{p}_{mt}_{hoff}")
                        kt = kk if hoff == 0 else kkb
                        qt = qaP if hoff == 0 else qbP
                        for c in range(2):
                            cs = slice(c * SC, (c + 1) * SC)
                            mm2(ps[:, cs],
                                kt[0:SK, m0:m0 + 128],
                                qt[0:SK, cs],
                                start=True, stop=True, second=(c == 1))
                        t_ = pt.tile([128, N], bf16, tag="t",
                                     name=f"t{p}_{mt}_{hoff}")
                        if (mt, hoff) in dve_evict:
                            if BIAS_FOLD:
                                # one-op square: DVE may read only one PSUM
                                # operand, so square via pow-by-2
                                nc.vector.tensor_single_scalar(
                                    t_[:], ps[:], 2.0, mybir.AluOpType.pow)
                            else:
                                tmp = pu.tile([128, N], bf16, tag="sqtmp",
                                              bufs=2,
                                              name=f"tmp{p}_{mt}_{hoff}")
                                nc.vector.tensor_scalar_add(tmp[:], ps[:],
                                                            40.0)
                                nc.vector.tensor_mul(t_[:], tmp[:], tmp[:])
                        else:
                            nc.scalar.activation(t_[:], ps[:], AF.Square,
                                                 bias=bias40[:], scale=1.0)
                        ts[(mt, hoff)] = t_
                    if mt == 0:
                        ups[0] = psu.tile([65, N], f32, tag="u", bufs=2,
                                          name=f"uA{p}")
                    elif mt == 1:
                        if p >= 2:
                            emit_norm_head(p - 2, 0)
                        ups[1] = psu.tile([65, N], f32, tag="u", bufs=2,
                                          name=f"uB{p}")
                    elif mt == 2 and p >= 2:
                        emit_norm_head(p - 2, 1)
                    if mt >= 2:
                        emit_av(0, mt - 2, 0)
                        emit_av(0, mt - 2, 1)
                    if mt >= 3:
                        emit_av(1, mt - 3, 0)
                        emit_av(1, mt - 3, 1)
                    if mt == 4 and p >= 1:
                        emit_recips(p - 1)
                    if p + 2 < 6:
                        if mt == 3:
                            emit_qtile(p + 2, 0)
                        elif mt == 6:
                            emit_qtile(p + 2, 2)
                # post-window leftovers
                emit_av(0, 6, 0)
                emit_av(0, 6, 1)
                emit_av(0, 7, 0)
                emit_av(0, 7, 1)
                finish_head(p, 0, ups[0], usbs, rgs)
                emit_av(1, 5, 0)
                emit_av(1, 5, 1)
                emit_av(1, 6, 0)
                emit_av(1, 6, 1)
                emit_av(1, 7, 0)
                emit_av(1, 7, 1)
                finish_head(p, 1, ups[1], usbs, rgs)

            # ---- tail: recips(5), normalize(4), proj interleaved with
            # normalize(5) so PE never waits on the last chain
            yps = [None] * CT

            def proj_head(ct, pool, tag):
                yps[ct] = pool.tile([128, N], f32, tag=tag, name=f"yps{ct}")
                for c in range(2):
                    cs = slice(c * SC, (c + 1) * SC)
                    for f in range(CT - 2):
                        mm(yps[ct][:, cs],
                           wp_sb[f][:, ct * 128:(ct + 1) * 128],
                           ot[f][:, cs],
                           start=(f == 0), stop=False)

            def proj_mid(ct):
                f = CT - 2
                for c in range(2):
                    cs = slice(c * SC, (c + 1) * SC)
                    mm2(yps[ct][:, cs],
                        wp_sb[f][:, ct * 128:(ct + 1) * 128],
                        ot[f][:, cs],
                        start=False, stop=False, second=(c == 1))

            def proj_last(ct):
                f = CT - 1
                ysb = pu.tile([128, N], ot_dt, tag="ysb", bufs=2,
                              name=f"ysb{ct}")
                for c in range(2):
                    cs = slice(c * SC, (c + 1) * SC)
                    mm2(yps[ct][:, cs],
                        wp_sb[f][:, ct * 128:(ct + 1) * 128],
                        ot[f][:, cs],
                        start=False, stop=True, second=(c == 1))
                    if TAIL_COPY_DVE and ct % 2 == 1:
                        nc.vector.tensor_copy(ysb[:, cs], yps[ct][:, cs])
                    else:
                        nc.scalar.copy(ysb[:, cs], yps[ct][:, cs])
                    if RING_V7:
                        # dispatch from ACT right after the producing copy:
                        # the dep is already satisfied, so no head-of-line
                        # stall, and the SP ring stays input-only
                        eng = nc.scalar
                    else:
                        eng = nc.sync if ct % 2 == 0 else nc.gpsimd
                    eng.dma_start(out=yt_d[ct * 128:(ct + 1) * 128, cs],
                                  in_=ysb[:, cs])

            # recips for pair 5 (its rg DMAs just issued above); pair-5
            # normalize right after the first proj_head so its bc matmuls
            # reach the PE as soon as the DVE recip chain lands
            emit_recips(5)

            emit_normalize(4)
            proj_head(0, psa, "st")
            emit_normalize(5)
            proj_head(1, psa, "st")
            proj_head(2, psu, "u")
            proj_head(3, psu, "u")
            for ct in range(4):
                proj_mid(ct)
            proj_last(0)
            proj_last(1)
            proj_head(4, psa, "st")
            proj_mid(4)
            proj_last(2)
            proj_head(5, psa, "st")
            proj_mid(5)
            proj_last(3)
            proj_last(4)
            proj_last(5)

            loop_stack.close()

    nc.compile()
    return nc


def _make_runner(nc):
    """Build the 8-core sharded jitted executable once (cached across calls)."""
    import jax
    import concourse.mybir as mybir
    from concourse import bass2jax
    from jax.experimental.shard_map import shard_map
    from jax.sharding import Mesh, PartitionSpec

    bass2jax.install_neuronx_cc_hook()
    partition_name = nc.partition_id_tensor.name if nc.partition_id_tensor else None
    in_names, out_names, out_avals, zero_outs = [], [], [], []
    for alloc in nc.m.functions[0].allocations:
        if not isinstance(alloc, mybir.MemoryLocationSet):
            continue
        name = alloc.memorylocations[0].name
        if alloc.kind == "ExternalInput":
            if name != partition_name:
                in_names.append(name)
        elif alloc.kind == "ExternalOutput":
            shape = tuple(alloc.tensor_shape)
            dtype = mybir.dt.np(alloc.dtype)
            out_names.append(name)
            out_avals.append(jax.core.ShapedArray(shape, dtype))
            zero_outs.append(np.zeros((B * shape[0], *shape[1:]), dtype))
    all_in_names = list(in_names) + list(out_names)
    if partition_name is not None:
        all_in_names.append(partition_name)

    def _body(*args):
        operands = list(args)
        if partition_name is not None:
            operands.append(bass2jax.partition_id_tensor())
        outs = bass2jax._bass_exec_p.bind(
            *operands,
            out_avals=tuple(out_avals),
            in_names=tuple(all_in_names),
            out_names=tuple(out_names),
            lowering_input_output_aliases=(),
            sim_require_finite=True,
            sim_require_nnan=True,
            nc=nc,
        )
        return tuple(outs)

    devices = jax.devices()[:B]
    mesh = Mesh(np.asarray(devices), ("core",))
    n_io = len(in_names) + len(out_avals)
    fn = jax.jit(shard_map(_body, mesh=mesh,
                           in_specs=(PartitionSpec("core"),) * n_io,
                           out_specs=(PartitionSpec("core"),) * len(out_avals),
                           check_rep=False))
    return fn, in_names, out_names, zero_outs


def kernel(x, qkv_w, proj_w, proj_b):
    global LAST_RESULT
    _ensure_path()
    import ml_dtypes

    bf16 = ml_dtypes.bfloat16
    x = np.asarray(x, dtype=np.float32)
    qkv_w = np.asarray(qkv_w, dtype=np.float32)
    proj_w = np.asarray(proj_w, dtype=np.float32)
    proj_b = np.asarray(proj_b, dtype=np.float32)

    if "runner" not in _CACHE:
        _CACHE["nc"] = _build_nc()
        _CACHE["runner"] = _make_runner(_CACHE["nc"])
    fn, in_names, out_names, zero_outs = _CACHE["runner"]

    wqk = np.ascontiguousarray(qkv_w[:2 * C].T).astype(bf16)
    wv = np.ascontiguousarray(qkv_w[2 * C:].T).astype(bf16)
    wp = np.ascontiguousarray(proj_w.T).astype(bf16)
    per_core = {
        "xt": np.concatenate(
            [np.ascontiguousarray(x[b].T).astype(bf16) for b in range(B)], axis=0),
        "wqk": np.concatenate([wqk] * B, axis=0),
        "wv": np.concatenate([wv] * B, axis=0),
        "wp": np.concatenate([wp] * B, axis=0),
    }
    args = [per_core[nm] for nm in in_names] + list(zero_outs)
    outs = fn(*args)
    yt = np.asarray(outs[out_names.index("yt")]).astype(np.float32)
    yt = yt.reshape(B, C, N)

    y = np.empty((B, N, C), dtype=np.float32)
    for b in range(B):
        y[b] = yt[b].T
    y += proj_b[None, None, :]
    return y



# revision 41
# speedup vs baseline: 1.4215x; 1.4215x over previous
"""Trainium2 Bass kernel for nn_Attention (Quad2-normalized multi-head attention).

Problem: B=8, N=1024, C=768, H=12 heads, head_dim=64.
  qkv = x @ qkv_w.T ; per head: s = q @ k.T ; t = (s/8 + 5)^2
  attn = t / rowsum(t) ; out = attn @ v ; y = out @ proj_w.T + proj_b
Sharding: data-parallel over batch B - one batch element per NeuronCore.

Layout (feature-major so PE contraction lands on partitions): xt = x[b].T
[768,1024] bf16; packed projections qkt = Wqk @ x.T in [feat, n]; V =
x @ Wv.T in [n, feat], ones-AUGMENTED (65 cols/head) so the AV matmul
(M=65) emits the row-sum r on psum row 64 for free; t = (s+40)^2 (the 64x
scale vs the reference cancels in the normalization).

HW-measured PE model this schedule is built around (slope-benchmarked on
the real device; the CoreSim cost model does NOT reflect these):
  - matmul cost ~= F*K/128 cycles at 2.4 GHz for K in (80,128], but K<=80
    falls into a 2 cycles-per-column slow mode (2x cost);
  - every matmul pays a serial ~107-124ns LDWEIGHTS tax (M columns /
    1.2GHz, never hidden, never deduped by this toolchain);
  - gpsimd (Q7) ops and SWDGE DMA dispatch are far slower than modeled -
    gpsimd is used only for memsets and a few yt DMAs.

Main techniques:
  - Scores contract at K=96 instead of the slow K=64: zero-padded offset-0
    operand copies qaP=[qA(64),0(32)], qbP=[qB(64),0(32)], kkb=[kB(64),
    junk(32)] are derived from the packed projection tiles by cheap SBUF
    DMAs + pad memsets (junk rows are cancelled by zero rows on the other
    operand). Head A uses kk[0:96] x qaP[0:96] directly.
  - The 1/r broadcast matmuls are also K=96: a one-hot [96,128] weights
    tile (row 0 / row 32 hot) against a zero-padded [96,N] recip tile
    replaces the K=1 ones-matmul (which ran in the slow mode).
  - Normalization of pair p is deferred TWO pairs (consumed at pair p+2):
    the serial chain usb-eviction -> r-row DMA (psum partition 64 ->
    rows {0,32} of the gather tile) -> batched DVE reciprocal -> bf16
    copy has a full window to complete and never stalls the PE. (Custom
    DVE ISA ops mis-address at partition offsets > 0, so the reciprocal
    always runs on an offset-0 AP.)
  - Both heads' AV matmuls interleave into the score window (A lags 2
    m-tiles, B lags 3); square evictions balanced ACT/DVE via DVE_EVICT.
  - PSUM (8 banks): "st" pool 2x[128,1024] for scores/projections/V plus
    "u" pool shared round-robin by ups0/ups1 (AV accumulators) and the
    bc broadcast tiles, in the order bc0(p-2), bc1(p-2), ups0(p), ups1(p).
  - proj splits head/mid/last and interleaves with the two tail
    normalizes so the PE stays fed to the end.
"""

import contextlib

import numpy as np

TRACE = False
TRACE_KWARGS = {}
LAST_RESULT = None
SIM_SAFE = False  # unused in v2 (no partially-initialized tiles remain)

B, N, C = 8, 1024, 768
H, HD = 12, 64
NT = N // 128      # 8 n/m tiles
CT = C // 128      # 6 feature tiles
SC = 512           # psum-bank chunk of the free dim
T_BUFS = 12
USB_BUFS = 6
# (mt, hoff) square evictions routed to DVE instead of ACT
DVE_EVICT = ((5, 0), (5, 64))

_CACHE = {}


def _ensure_path():
    import sys
    for p in ("/opt/trn_rl_repo", "/root/.axon_site/_ro/trn_rl_repo"):
        if p not in sys.path:
            sys.path.insert(0, p)


def _build_nc(loop_n=None):
    import concourse.bacc as bacc
    import concourse.mybir as mybir
    import concourse.tile as tile

    f32 = mybir.dt.float32
    bf16 = mybir.dt.bfloat16
    AF = mybir.ActivationFunctionType

    nc = bacc.Bacc("TRN2", target_bir_lowering=False)
    xt_d = nc.dram_tensor("xt", [C, N], bf16, kind="ExternalInput")
    wqk_d = nc.dram_tensor("wqk", [C, 2 * C], bf16, kind="ExternalInput")
    wv_d = nc.dram_tensor("wv", [C, C], bf16, kind="ExternalInput")
    wp_d = nc.dram_tensor("wp", [C, C], bf16, kind="ExternalInput")
    yt_d = nc.dram_tensor("yt", [C, N], f32, kind="ExternalOutput")

    with tile.TileContext(nc) as tc:
        with (
            tc.tile_pool(name="pw", bufs=1) as pw,
            tc.tile_pool(name="pq", bufs=1) as pq,
            tc.tile_pool(name="pt", bufs=T_BUFS) as pt,
            tc.tile_pool(name="pu", bufs=2) as pu,
            tc.tile_pool(name="psa", bufs=2, space="PSUM") as psa,
            tc.tile_pool(name="psu", bufs=2, space="PSUM") as psu,
        ):
            mm = nc.tensor.matmul

            bias40 = pw.tile([128, 1], f32, tag="bias40", name="bias40")
            ones_bc = pw.tile([96, 128], bf16, tag="ones_bc", name="ones_bc")
            nc.gpsimd.memset(bias40[:], 40.0)
            nc.gpsimd.memset(ones_bc[:], 0.0)
            nc.gpsimd.memset(ones_bc[0:1, 0:64], 1.0)
            nc.gpsimd.memset(ones_bc[32:33, 64:128], 1.0)

            loop_ctx = tc.For_i(0, loop_n, 1) if loop_n else contextlib.nullcontext()
            loop_stack = contextlib.ExitStack()
            loop_stack.enter_context(loop_ctx)

            # ---- input DMAs, first-use order, alternating queues
            wv_sb = []
            xt_sb = []
            wqk_sb = []
            wp_sb = []
            for k in range(CT):
                t_ = pw.tile([128, N], bf16, tag=f"xt{k}", name=f"xt{k}")
                nc.sync.dma_start(out=t_[:], in_=xt_d[k * 128:(k + 1) * 128, :])
                xt_sb.append(t_)
                t_ = pw.tile([128, C], bf16, tag=f"wv{k}", name=f"wv{k}")
                nc.sync.dma_start(out=t_[:], in_=wv_d[k * 128:(k + 1) * 128, :])
                wv_sb.append(t_)
            for k in range(CT):
                t_ = pw.tile([128, 2 * C], bf16, tag=f"wqk{k}", name=f"wqk{k}")
                nc.sync.dma_start(out=t_[:], in_=wqk_d[k * 128:(k + 1) * 128, :])
                wqk_sb.append(t_)
            for k in range(CT):
                t_ = pw.tile([128, C], bf16, tag=f"wp{k}", name=f"wp{k}")
                nc.sync.dma_start(out=t_[:], in_=wp_d[k * 128:(k + 1) * 128, :])
                wp_sb.append(t_)

            # ---- V = x @ Wv.T, [n, feat] layout, ones-augmented (65 cols/head,
            # ones at 65h+64) so AV's M=65 also produces the row-sum r
            vv = []
            for mt in range(NT):
                ps = psa.tile([128, C], f32, tag="st", name=f"vps{mt}")
                for (o, w) in ((0, SC), (SC, C - SC)):
                    for k in range(CT):
                        mm(ps[:, o:o + w],
                           xt_sb[k][:, mt * 128:(mt + 1) * 128],
                           wv_sb[k][:, o:o + w],
                           start=(k == 0), stop=(k == CT - 1))
                v_ = pq.tile([128, H * 65], bf16, tag=f"vv{mt}", name=f"vv{mt}")
                nc.gpsimd.memset(v_[:], 1.0)
                vdst = v_[:, :].rearrange("p (h s) -> p h s", s=65)[:, :, 0:64]
                vsrc = ps[:, 0:C].rearrange("p (h s) -> p h s", s=64)
                nc.vector.tensor_copy(vdst, vsrc)
                vv.append(v_)

            # ---- QK^T = Wqk @ x.T, [feat, n] layout, packed projections
            # (pair p: Q tile = heads 2p,2p+1 stacked; K tile likewise).
            # Scores contract over K=96 because the PE runs a 2x-slower mode
            # for K<=65: each score operand gets a zero-padded offset-0 copy
            # ([data(64), 0(32)]) derived by one SBUF DMA + a memset of the
            # pad rows. qaP/qbP pads zero the junk rows of the kk/kkb lhsT.
            qkt = {}

            def emit_qtile(p, which):
                # which: 0 = packed Q tile, 2 = packed K tile
                col0 = 128 * p + (C if which == 2 else 0)
                ps = psa.tile([128, N], f32, tag="st", name=f"qkps{p}_{which}")
                for c in range(2):
                    for k in range(CT):
                        mm(ps[:, c * SC:(c + 1) * SC],
                           wqk_sb[k][:, col0:col0 + 128],
                           xt_sb[k][:, c * SC:(c + 1) * SC],
                           start=(k == 0), stop=(k == CT - 1))
                q_ = pq.tile([128, N], bf16, tag=f"qkt{which}", bufs=2,
                             name=f"qkt{p}_{which}")
                nc.vector.tensor_copy(q_[:], ps[:])
                qkt[(p, which)] = q_
                if which == 0:
                    for hi, tag in ((0, "qaP"), (1, "qbP")):
                        t_ = pq.tile([96, N], bf16, tag=tag, bufs=3,
                                     name=f"{tag}{p}")
                        nc.gpsimd.memset(t_[64:96, :], 0.0)
                        nc.sync.dma_start(out=t_[0:64, :],
                                          in_=q_[64 * hi:64 * hi + 64, :])
                        qkt[(p, 3 + hi)] = t_
                else:
                    t_ = pq.tile([96, N], bf16, tag="kkb", bufs=3,
                                 name=f"kkb{p}")
                    nc.gpsimd.memset(t_[64:96, :], 0.0)
                    nc.sync.dma_start(out=t_[0:64, :], in_=q_[64:128, :])
                    qkt[(p, 5)] = t_

            for p0 in (0, 1):
                emit_qtile(p0, 0)
                emit_qtile(p0, 2)

            # ---- attention pairs. State carried across pairs:
            # fin[p] = {"usbs", "rgs", "rrs"} after pair p's AV completes;
            # rrs[hi] bf16 [1, N] ready ~one pair later; normalize(p) at p+2.
            ot = [None] * 6
            fin = [None] * 6

            def finish_head(p, hi, ups_hi, usbs, rgs):
                # U+r eviction; r row (psum partition 64) -> partition {0,32}
                # of the pair's gather tile; recip batched in the NEXT window
                usb = pu.tile([65, N], f32, tag="usb", bufs=USB_BUFS,
                              name=f"usb{p}_{hi}")
                nc.vector.tensor_copy(usb[:], ups_hi[:])
                if hi == 0:
                    rgs[0] = pu.tile([33, N], f32, tag="rg", bufs=2,
                                     name=f"rg{p}")
                    # rows 1:31 are never DMA-written; the K=96 bcast reads
                    # their reciprocals against zero weights, and 0*Inf/NaN
                    # would poison the accumulation - keep them finite
                    nc.gpsimd.memset(rgs[0][:], 1.0)
                row = hi * 32
                nc.sync.dma_start(out=rgs[0][row:row + 1, :],
                                  in_=usb[64:65, :])
                usbs[hi] = usb

            def emit_recips(p):
                # pair p's batched reciprocal, emitted mid-window of pair p+1
                rg = fin[p]["rgs"][0]
                rrf = pu.tile([33, N], f32, tag="rrf", bufs=2, name=f"rrf{p}")
                nc.vector.reciprocal_approx_fast(rrf[:, :], rg[:, :])
                rr = pu.tile([96, N], bf16, tag="rr", bufs=2, name=f"rr{p}")
                nc.gpsimd.memset(rr[32:64, :], 0.0)
                nc.gpsimd.memset(rr[64:96, :], 0.0)
                nc.vector.tensor_copy(rr[0:33, :], rrf[:, :])
                fin[p]["rrs"][0] = rr
                fin[p]["rrs"][1] = rr

            def alloc_bc(p):
                # slot-order bookkeeping: bc tiles must take "u" slots before
                # the new pair's ups0/ups1 (round-robin per tag)
                fin[p]["bcs"] = [
                    psu.tile([64, N], f32, tag="u", name=f"bc{p}_{hi}")
                    for hi in (0, 1)
                ]

            def emit_norm_head(p, hi):
                # pair p's normalize for one head: PE ones-bcast of 1/r,
                # then the DVE mul; odd head lands in ot via SBUF DMA
                usbs, rrs = fin[p]["usbs"], fin[p]["rrs"]
                bc = fin[p]["bcs"][hi]
                for c in range(2):
                    cs = slice(c * SC, (c + 1) * SC)
                    mm(bc[:, cs], ones_bc[0:96, 64 * hi:64 * hi + 64],
                       rrs[hi][0:96, cs],
                       start=True, stop=True)
                if hi == 0:
                    o_ = pq.tile([128, N], bf16, tag=f"ot{p}", name=f"ot{p}")
                    ot[p] = o_
                    nc.vector.tensor_mul(o_[0:64, :], usbs[0][0:64, :],
                                         bc[:, :])
                else:
                    o_ = ot[p]
                    ottmp = pu.tile([64, N], bf16, tag="ottmp", bufs=2,
                                    name=f"ottmp{p}")
                    nc.vector.tensor_mul(ottmp[:, :], usbs[1][0:64, :],
                                         bc[:, :])
                    nc.sync.dma_start(out=o_[64:128, :], in_=ottmp[:, :])

            def emit_normalize(p):
                alloc_bc(p)
                emit_norm_head(p, 0)
                emit_norm_head(p, 1)

            for p in range(6):
                kk = qkt[(p, 2)]
                qaP = qkt[(p, 3)]
                qbP = qkt[(p, 4)]
                kkb = qkt[(p, 5)]
                usbs = {}
                rgs = {}
                ts = {}
                ups = {}
                fin[p] = {"usbs": usbs, "rgs": rgs, "rrs": {}}

                def emit_av(hi, k, c):
                    cs = slice(c * SC, (c + 1) * SC)
                    h = 2 * p + hi
                    mm(ups[hi][:, cs],
                       vv[k][:, h * 65:h * 65 + 65],
                       ts[(k, hi * 64)][:, cs],
                       start=(k == 0), stop=(k == NT - 1))

                for mt in range(NT):
                    if mt == 0 and p >= 2:
                        alloc_bc(p - 2)
                    # scores at K=96 (K<=65 runs a 2x-slower PE mode):
                    # head A: kk[0:96]  x qaP[0:96]  (qaP rows 64:96 zero)
                    # head B: kkb[0:96] x qbP[0:96]  (kkb rows 64:96 zero)
                    m0 = mt * 128
                    for hoff in (0, 64):
                        ps = psa.tile([128, N], f32, tag="st",
                                      name=f"sps{p}_{mt}_{hoff}")
                        kt = kk if hoff == 0 else kkb
                        qt = qaP if hoff == 0 else qbP
                        for c in range(2):
                            cs = slice(c * SC, (c + 1) * SC)
                            mm(ps[:, cs],
                               kt[0:96, m0:m0 + 128],
                               qt[0:96, cs],
                               start=True, stop=True)
                        t_ = pt.tile([128, N], bf16, tag="t",
                                     name=f"t{p}_{mt}_{hoff}")
                        if (mt, hoff) in DVE_EVICT:
                            tmp = pu.tile([128, N], bf16, tag="sqtmp", bufs=2,
                                          name=f"tmp{p}_{mt}_{hoff}")
                            nc.vector.tensor_scalar_add(tmp[:], ps[:], 40.0)
                            nc.vector.tensor_mul(t_[:], tmp[:], tmp[:])
                        else:
                            nc.scalar.activation(t_[:], ps[:], AF.Square,
                                                 bias=bias40[:], scale=1.0)
                        ts[(mt, hoff)] = t_
                    if mt == 0:
                        ups[0] = psu.tile([65, N], f32, tag="u", bufs=2,
                                          name=f"uA{p}")
                    elif mt == 1:
                        if p >= 2:
                            emit_norm_head(p - 2, 0)
                        ups[1] = psu.tile([65, N], f32, tag="u", bufs=2,
                                          name=f"uB{p}")
                    elif mt == 2 and p >= 2:
                        emit_norm_head(p - 2, 1)
                    if mt >= 2:
                        emit_av(0, mt - 2, 0)
                        emit_av(0, mt - 2, 1)
                    if mt >= 3:
                        emit_av(1, mt - 3, 0)
                        emit_av(1, mt - 3, 1)
                    if mt == 4 and p >= 1:
                        emit_recips(p - 1)
                    if p + 2 < 6:
                        if mt == 3:
                            emit_qtile(p + 2, 0)
                        elif mt == 6:
                            emit_qtile(p + 2, 2)
                # post-window leftovers
                emit_av(0, 6, 0)
                emit_av(0, 6, 1)
                emit_av(0, 7, 0)
                emit_av(0, 7, 1)
                finish_head(p, 0, ups[0], usbs, rgs)
                emit_av(1, 5, 0)
                emit_av(1, 5, 1)
                emit_av(1, 6, 0)
                emit_av(1, 6, 1)
                emit_av(1, 7, 0)
                emit_av(1, 7, 1)
                finish_head(p, 1, ups[1], usbs, rgs)

            # ---- tail: recips(5), normalize(4), proj interleaved with
            # normalize(5) so PE never waits on the last chain
            yps = [None] * CT

            def proj_head(ct, pool, tag):
                yps[ct] = pool.tile([128, N], f32, tag=tag, name=f"yps{ct}")
                for c in range(2):
                    cs = slice(c * SC, (c + 1) * SC)
                    for f in range(CT - 2):
                        mm(yps[ct][:, cs],
                           wp_sb[f][:, ct * 128:(ct + 1) * 128],
                           ot[f][:, cs],
                           start=(f == 0), stop=False)

            def proj_mid(ct):
                f = CT - 2
                for c in range(2):
                    cs = slice(c * SC, (c + 1) * SC)
                    mm(yps[ct][:, cs],
                       wp_sb[f][:, ct * 128:(ct + 1) * 128],
                       ot[f][:, cs],
                       start=False, stop=False)

            def proj_last(ct):
                f = CT - 1
                ysb = pu.tile([128, N], f32, tag="ysb", bufs=2, name=f"ysb{ct}")
                for c in range(2):
                    cs = slice(c * SC, (c + 1) * SC)
                    mm(yps[ct][:, cs],
                       wp_sb[f][:, ct * 128:(ct + 1) * 128],
                       ot[f][:, cs],
                       start=False, stop=True)
                    nc.scalar.copy(ysb[:, cs], yps[ct][:, cs])
                    eng = nc.sync if ct % 2 == 0 else nc.gpsimd
                    eng.dma_start(out=yt_d[ct * 128:(ct + 1) * 128, cs],
                                  in_=ysb[:, cs])

            # recips for pair 5 (its rg DMAs just issued above)
            emit_recips(5)

            emit_normalize(4)
            proj_head(0, psa, "st")
            proj_head(1, psa, "st")
            emit_normalize(5)
            proj_head(2, psu, "u")
            proj_head(3, psu, "u")
            for ct in range(4):
                proj_mid(ct)
            proj_last(0)
            proj_last(1)
            proj_head(4, psa, "st")
            proj_mid(4)
            proj_last(2)
            proj_head(5, psa, "st")
            proj_mid(5)
            proj_last(3)
            proj_last(4)
            proj_last(5)

            loop_stack.close()

    nc.compile()
    return nc


def _make_runner(nc):
    """Build the 8-core sharded jitted executable once (cached across calls)."""
    import jax
    import concourse.mybir as mybir
    from concourse import bass2jax
    from jax.experimental.shard_map import shard_map
    from jax.sharding import Mesh, PartitionSpec

    bass2jax.install_neuronx_cc_hook()
    partition_name = nc.partition_id_tensor.name if nc.partition_id_tensor else None
    in_names, out_names, out_avals, zero_outs = [], [], [], []
    for alloc in nc.m.functions[0].allocations:
        if not isinstance(alloc, mybir.MemoryLocationSet):
            continue
        name = alloc.memorylocations[0].name
        if alloc.kind == "ExternalInput":
            if name != partition_name:
                in_names.append(name)
        elif alloc.kind == "ExternalOutput":
            shape = tuple(alloc.tensor_shape)
            dtype = mybir.dt.np(alloc.dtype)
            out_names.append(name)
            out_avals.append(jax.core.ShapedArray(shape, dtype))
            zero_outs.append(np.zeros((B * shape[0], *shape[1:]), dtype))
    all_in_names = list(in_names) + list(out_names)
    if partition_name is not None:
        all_in_names.append(partition_name)

    def _body(*args):
        operands = list(args)
        if partition_name is not None:
            operands.append(bass2jax.partition_id_tensor())
        outs = bass2jax._bass_exec_p.bind(
            *operands,
            out_avals=tuple(out_avals),
            in_names=tuple(all_in_names),
            out_names=tuple(out_names),
            lowering_input_output_aliases=(),
            sim_require_finite=True,
            sim_require_nnan=True,
            nc=nc,
        )
        return tuple(outs)

    devices = jax.devices()[:B]
    mesh = Mesh(np.asarray(devices), ("core",))
    n_io = len(in_names) + len(out_avals)
    fn = jax.jit(shard_map(_body, mesh=mesh,
                           in_specs=(PartitionSpec("core"),) * n_io,
                           out_specs=(PartitionSpec("core"),) * len(out_avals),
                           check_rep=False))
    return fn, in_names, out_names, zero_outs


def kernel(x, qkv_w, proj_w, proj_b):
    global LAST_RESULT
    _ensure_path()
    import ml_dtypes

    bf16 = ml_dtypes.bfloat16
    x = np.asarray(x, dtype=np.float32)
    qkv_w = np.asarray(qkv_w, dtype=np.float32)
    proj_w = np.asarray(proj_w, dtype=np.float32)
    proj_b = np.asarray(proj_b, dtype=np.float32)

    if "runner" not in _CACHE:
        _CACHE["nc"] = _build_nc()
        _CACHE["runner"] = _make_runner(_CACHE["nc"])
    fn, in_names, out_names, zero_outs = _CACHE["runner"]

    wqk = np.ascontiguousarray(qkv_w[:2 * C].T).astype(bf16)
    wv = np.ascontiguousarray(qkv_w[2 * C:].T).astype(bf16)
    wp = np.ascontiguousarray(proj_w.T).astype(bf16)
    per_core = {
        "xt": np.concatenate(
            [np.ascontiguousarray(x[b].T).astype(bf16) for b in range(B)], axis=0),
        "wqk": np.concatenate([wqk] * B, axis=0),
        "wv": np.concatenate([wv] * B, axis=0),
        "wp": np.concatenate([wp] * B, axis=0),
    }
    args = [per_core[nm] for nm in in_names] + list(zero_outs)
    outs = fn(*args)
    yt = np.asarray(outs[out_names.index("yt")]).reshape(B, C, N)

    y = np.empty((B, N, C), dtype=np.float32)
    for b in range(B):
        y[b] = yt[b].T
    y += proj_b[None, None, :]
    return y



# revision 42
# speedup vs baseline: 1.4350x; 1.0095x over previous
"""Trainium2 Bass kernel for nn_Attention (Quad2-normalized multi-head attention).

Problem: B=8, N=1024, C=768, H=12 heads, head_dim=64.
  qkv = x @ qkv_w.T ; per head: s = q @ k.T ; t = (s/8 + 5)^2
  attn = t / rowsum(t) ; out = attn @ v ; y = out @ proj_w.T + proj_b
Sharding: data-parallel over batch B - one batch element per NeuronCore.

Layout (feature-major so PE contraction lands on partitions): xt = x[b].T
[768,1024] bf16; packed projections qkt = Wqk @ x.T in [feat, n]; V =
x @ Wv.T in [n, feat], ones-AUGMENTED (65 cols/head) so the AV matmul
(M=65) emits the row-sum r on psum row 64 for free; t = (s+40)^2 (the 64x
scale vs the reference cancels in the normalization).

HW-measured PE model this schedule is built around (slope-benchmarked on
the real device; the CoreSim cost model does NOT reflect these):
  - matmul cost ~= F*K/128 cycles at 2.4 GHz for K in (80,128], but K<=80
    falls into a 2 cycles-per-column slow mode (2x cost);
  - every matmul pays a serial ~107-124ns LDWEIGHTS tax (M columns /
    1.2GHz, never hidden, never deduped by this toolchain);
  - gpsimd (Q7) ops and SWDGE DMA dispatch are far slower than modeled -
    gpsimd is used only for memsets and a few yt DMAs.

Main techniques:
  - Scores contract at K=96 instead of the slow K=64: zero-padded offset-0
    operand copies qaP=[qA(64),0(32)], qbP=[qB(64),0(32)], kkb=[kB(64),
    junk(32)] are derived from the packed projection tiles by cheap SBUF
    DMAs + pad memsets (junk rows are cancelled by zero rows on the other
    operand). Head A uses kk[0:96] x qaP[0:96] directly.
  - The 1/r broadcast matmuls are also K=96: a one-hot [96,128] weights
    tile (row 0 / row 32 hot) against a zero-padded [96,N] recip tile
    replaces the K=1 ones-matmul (which ran in the slow mode).
  - Normalization of pair p is deferred TWO pairs (consumed at pair p+2):
    the serial chain usb-eviction -> r-row DMA (psum partition 64 ->
    rows {0,32} of the gather tile) -> batched DVE reciprocal -> bf16
    copy has a full window to complete and never stalls the PE. (Custom
    DVE ISA ops mis-address at partition offsets > 0, so the reciprocal
    always runs on an offset-0 AP.)
  - Both heads' AV matmuls interleave into the score window (A lags 2
    m-tiles, B lags 3); square evictions balanced ACT/DVE via DVE_EVICT.
  - PSUM (8 banks): "st" pool 2x[128,1024] for scores/projections/V plus
    "u" pool shared round-robin by ups0/ups1 (AV accumulators) and the
    bc broadcast tiles, in the order bc0(p-2), bc1(p-2), ups0(p), ups1(p).
  - proj splits head/mid/last and interleaves with the two tail
    normalizes so the PE stays fed to the end.
"""

import contextlib

import numpy as np

TRACE = False
TRACE_KWARGS = {}
LAST_RESULT = None
SIM_SAFE = False  # unused in v2 (no partially-initialized tiles remain)

B, N, C = 8, 1024, 768
H, HD = 12, 64
NT = N // 128      # 8 n/m tiles
CT = C // 128      # 6 feature tiles
SC = 512           # psum-bank chunk of the free dim
T_BUFS = 12
USB_BUFS = 6
# (mt, hoff) square evictions routed to DVE instead of ACT
DVE_EVICT = ((5, 0), (5, 64))

_CACHE = {}


def _ensure_path():
    import sys
    for p in ("/opt/trn_rl_repo", "/root/.axon_site/_ro/trn_rl_repo"):
        if p not in sys.path:
            sys.path.insert(0, p)


def _build_nc(loop_n=None):
    import concourse.bacc as bacc
    import concourse.mybir as mybir
    import concourse.tile as tile

    f32 = mybir.dt.float32
    bf16 = mybir.dt.bfloat16
    AF = mybir.ActivationFunctionType

    nc = bacc.Bacc("TRN2", target_bir_lowering=False)
    xt_d = nc.dram_tensor("xt", [C, N], bf16, kind="ExternalInput")
    wqk_d = nc.dram_tensor("wqk", [C, 2 * C], bf16, kind="ExternalInput")
    wv_d = nc.dram_tensor("wv", [C, C], bf16, kind="ExternalInput")
    wp_d = nc.dram_tensor("wp", [C, C], bf16, kind="ExternalInput")
    yt_d = nc.dram_tensor("yt", [C, N], f32, kind="ExternalOutput")

    with tile.TileContext(nc) as tc:
        with (
            tc.tile_pool(name="pw", bufs=1) as pw,
            tc.tile_pool(name="pq", bufs=1) as pq,
            tc.tile_pool(name="pt", bufs=T_BUFS) as pt,
            tc.tile_pool(name="pu", bufs=2) as pu,
            tc.tile_pool(name="psa", bufs=2, space="PSUM") as psa,
            tc.tile_pool(name="psu", bufs=2, space="PSUM") as psu,
        ):
            mm = nc.tensor.matmul

            bias40 = pw.tile([128, 1], f32, tag="bias40", name="bias40")
            ones_bc = pw.tile([96, 128], bf16, tag="ones_bc", name="ones_bc")
            nc.gpsimd.memset(bias40[:], 40.0)
            nc.gpsimd.memset(ones_bc[:], 0.0)
            nc.gpsimd.memset(ones_bc[0:1, 0:64], 1.0)
            nc.gpsimd.memset(ones_bc[32:33, 64:128], 1.0)

            loop_ctx = tc.For_i(0, loop_n, 1) if loop_n else contextlib.nullcontext()
            loop_stack = contextlib.ExitStack()
            loop_stack.enter_context(loop_ctx)

            # ---- input DMAs, first-use order, alternating queues
            wv_sb = []
            xt_sb = []
            wqk_sb = []
            wp_sb = []
            for k in range(CT):
                t_ = pw.tile([128, N], bf16, tag=f"xt{k}", name=f"xt{k}")
                nc.sync.dma_start(out=t_[:], in_=xt_d[k * 128:(k + 1) * 128, :])
                xt_sb.append(t_)
                t_ = pw.tile([128, C], bf16, tag=f"wv{k}", name=f"wv{k}")
                nc.sync.dma_start(out=t_[:], in_=wv_d[k * 128:(k + 1) * 128, :])
                wv_sb.append(t_)
            for k in range(CT):
                t_ = pw.tile([128, 2 * C], bf16, tag=f"wqk{k}", name=f"wqk{k}")
                nc.sync.dma_start(out=t_[:], in_=wqk_d[k * 128:(k + 1) * 128, :])
                wqk_sb.append(t_)
            for k in range(CT):
                t_ = pw.tile([128, C], bf16, tag=f"wp{k}", name=f"wp{k}")
                nc.sync.dma_start(out=t_[:], in_=wp_d[k * 128:(k + 1) * 128, :])
                wp_sb.append(t_)

            # ---- V = x @ Wv.T, [n, feat] layout, ones-augmented (65 cols/head,
            # ones at 65h+64) so AV's M=65 also produces the row-sum r
            vv = []
            for mt in range(NT):
                ps = psa.tile([128, C], f32, tag="st", name=f"vps{mt}")
                for (o, w) in ((0, SC), (SC, C - SC)):
                    for k in range(CT):
                        mm(ps[:, o:o + w],
                           xt_sb[k][:, mt * 128:(mt + 1) * 128],
                           wv_sb[k][:, o:o + w],
                           start=(k == 0), stop=(k == CT - 1))
                v_ = pq.tile([128, H * 65], bf16, tag=f"vv{mt}", name=f"vv{mt}")
                nc.gpsimd.memset(v_[:], 1.0)
                vdst = v_[:, :].rearrange("p (h s) -> p h s", s=65)[:, :, 0:64]
                vsrc = ps[:, 0:C].rearrange("p (h s) -> p h s", s=64)
                nc.vector.tensor_copy(vdst, vsrc)
                vv.append(v_)

            # ---- QK^T = Wqk @ x.T, [feat, n] layout, packed projections
            # (pair p: Q tile = heads 2p,2p+1 stacked; K tile likewise).
            # Scores contract over K=96 because the PE runs a 2x-slower mode
            # for K<=65: each score operand gets a zero-padded offset-0 copy
            # ([data(64), 0(32)]) derived by one SBUF DMA + a memset of the
            # pad rows. qaP/qbP pads zero the junk rows of the kk/kkb lhsT.
            qkt = {}

            def emit_qtile(p, which):
                # which: 0 = packed Q tile, 2 = packed K tile
                col0 = 128 * p + (C if which == 2 else 0)
                ps = psa.tile([128, N], f32, tag="st", name=f"qkps{p}_{which}")
                for c in range(2):
                    for k in range(CT):
                        mm(ps[:, c * SC:(c + 1) * SC],
                           wqk_sb[k][:, col0:col0 + 128],
                           xt_sb[k][:, c * SC:(c + 1) * SC],
                           start=(k == 0), stop=(k == CT - 1))
                q_ = pq.tile([128, N], bf16, tag=f"qkt{which}", bufs=2,
                             name=f"qkt{p}_{which}")
                nc.vector.tensor_copy(q_[:], ps[:])
                qkt[(p, which)] = q_
                if which == 0:
                    for hi, tag in ((0, "qaP"), (1, "qbP")):
                        t_ = pq.tile([96, N], bf16, tag=tag, bufs=3,
                                     name=f"{tag}{p}")
                        nc.gpsimd.memset(t_[64:96, :], 0.0)
                        nc.sync.dma_start(out=t_[0:64, :],
                                          in_=q_[64 * hi:64 * hi + 64, :])
                        qkt[(p, 3 + hi)] = t_
                else:
                    t_ = pq.tile([96, N], bf16, tag="kkb", bufs=3,
                                 name=f"kkb{p}")
                    nc.gpsimd.memset(t_[64:96, :], 0.0)
                    nc.sync.dma_start(out=t_[0:64, :], in_=q_[64:128, :])
                    qkt[(p, 5)] = t_

            for p0 in (0, 1):
                emit_qtile(p0, 0)
                emit_qtile(p0, 2)

            # ---- attention pairs. State carried across pairs:
            # fin[p] = {"usbs", "rgs", "rrs"} after pair p's AV completes;
            # rrs[hi] bf16 [1, N] ready ~one pair later; normalize(p) at p+2.
            ot = [None] * 6
            fin = [None] * 6

            def finish_head(p, hi, ups_hi, usbs, rgs):
                # U+r eviction; r row (psum partition 64) -> partition {0,32}
                # of the pair's gather tile; recip batched in the NEXT window
                usb = pu.tile([65, N], f32, tag="usb", bufs=USB_BUFS,
                              name=f"usb{p}_{hi}")
                nc.vector.tensor_copy(usb[:], ups_hi[:])
                if hi == 0:
                    rgs[0] = pu.tile([33, N], f32, tag="rg", bufs=2,
                                     name=f"rg{p}")
                    # rows 1:31 are never DMA-written; the K=96 bcast reads
                    # their reciprocals against zero weights, and 0*Inf/NaN
                    # would poison the accumulation - keep them finite
                    nc.gpsimd.memset(rgs[0][:], 1.0)
                row = hi * 32
                nc.sync.dma_start(out=rgs[0][row:row + 1, :],
                                  in_=usb[64:65, :])
                usbs[hi] = usb

            def emit_recips(p):
                # pair p's batched reciprocal, emitted mid-window of pair p+1
                rg = fin[p]["rgs"][0]
                rrf = pu.tile([33, N], f32, tag="rrf", bufs=2, name=f"rrf{p}")
                nc.vector.reciprocal_approx_fast(rrf[:, :], rg[:, :])
                rr = pu.tile([96, N], bf16, tag="rr", bufs=2, name=f"rr{p}")
                nc.gpsimd.memset(rr[32:64, :], 0.0)
                nc.gpsimd.memset(rr[64:96, :], 0.0)
                nc.vector.tensor_copy(rr[0:33, :], rrf[:, :])
                fin[p]["rrs"][0] = rr
                fin[p]["rrs"][1] = rr

            def alloc_bc(p):
                # slot-order bookkeeping: bc tiles must take "u" slots before
                # the new pair's ups0/ups1 (round-robin per tag)
                fin[p]["bcs"] = [
                    psu.tile([64, N], f32, tag="u", name=f"bc{p}_{hi}")
                    for hi in (0, 1)
                ]

            def emit_norm_head(p, hi):
                # pair p's normalize for one head: PE ones-bcast of 1/r,
                # then the DVE mul; odd head lands in ot via SBUF DMA
                usbs, rrs = fin[p]["usbs"], fin[p]["rrs"]
                bc = fin[p]["bcs"][hi]
                for c in range(2):
                    cs = slice(c * SC, (c + 1) * SC)
                    mm(bc[:, cs], ones_bc[0:96, 64 * hi:64 * hi + 64],
                       rrs[hi][0:96, cs],
                       start=True, stop=True)
                if hi == 0:
                    o_ = pq.tile([128, N], bf16, tag=f"ot{p}", name=f"ot{p}")
                    ot[p] = o_
                    nc.vector.tensor_mul(o_[0:64, :], usbs[0][0:64, :],
                                         bc[:, :])
                else:
                    o_ = ot[p]
                    ottmp = pu.tile([64, N], bf16, tag="ottmp", bufs=2,
                                    name=f"ottmp{p}")
                    nc.vector.tensor_mul(ottmp[:, :], usbs[1][0:64, :],
                                         bc[:, :])
                    nc.sync.dma_start(out=o_[64:128, :], in_=ottmp[:, :])

            def emit_normalize(p):
                alloc_bc(p)
                emit_norm_head(p, 0)
                emit_norm_head(p, 1)

            for p in range(6):
                kk = qkt[(p, 2)]
                qaP = qkt[(p, 3)]
                qbP = qkt[(p, 4)]
                kkb = qkt[(p, 5)]
                usbs = {}
                rgs = {}
                ts = {}
                ups = {}
                fin[p] = {"usbs": usbs, "rgs": rgs, "rrs": {}}

                def emit_av(hi, k, c):
                    cs = slice(c * SC, (c + 1) * SC)
                    h = 2 * p + hi
                    mm(ups[hi][:, cs],
                       vv[k][:, h * 65:h * 65 + 65],
                       ts[(k, hi * 64)][:, cs],
                       start=(k == 0), stop=(k == NT - 1))

                for mt in range(NT):
                    if mt == 0 and p >= 2:
                        alloc_bc(p - 2)
                    # scores at K=96 (K<=65 runs a 2x-slower PE mode):
                    # head A: kk[0:96]  x qaP[0:96]  (qaP rows 64:96 zero)
                    # head B: kkb[0:96] x qbP[0:96]  (kkb rows 64:96 zero)
                    m0 = mt * 128
                    for hoff in (0, 64):
                        ps = psa.tile([128, N], f32, tag="st",
                                      name=f"sps{p}_{mt}_{hoff}")
                        kt = kk if hoff == 0 else kkb
                        qt = qaP if hoff == 0 else qbP
                        for c in range(2):
                            cs = slice(c * SC, (c + 1) * SC)
                            mm(ps[:, cs],
                               kt[0:96, m0:m0 + 128],
                               qt[0:96, cs],
                               start=True, stop=True)
                        t_ = pt.tile([128, N], bf16, tag="t",
                                     name=f"t{p}_{mt}_{hoff}")
                        if (mt, hoff) in DVE_EVICT:
                            tmp = pu.tile([128, N], bf16, tag="sqtmp", bufs=2,
                                          name=f"tmp{p}_{mt}_{hoff}")
                            nc.vector.tensor_scalar_add(tmp[:], ps[:], 40.0)
                            nc.vector.tensor_mul(t_[:], tmp[:], tmp[:])
                        else:
                            nc.scalar.activation(t_[:], ps[:], AF.Square,
                                                 bias=bias40[:], scale=1.0)
                        ts[(mt, hoff)] = t_
                    if mt == 0:
                        ups[0] = psu.tile([65, N], f32, tag="u", bufs=2,
                                          name=f"uA{p}")
                    elif mt == 1:
                        if p >= 2:
                            emit_norm_head(p - 2, 0)
                        ups[1] = psu.tile([65, N], f32, tag="u", bufs=2,
                                          name=f"uB{p}")
                    elif mt == 2 and p >= 2:
                        emit_norm_head(p - 2, 1)
                    if mt >= 2:
                        emit_av(0, mt - 2, 0)
                        emit_av(0, mt - 2, 1)
                    if mt >= 3:
                        emit_av(1, mt - 3, 0)
                        emit_av(1, mt - 3, 1)
                    if mt == 4 and p >= 1:
                        emit_recips(p - 1)
                    if p + 2 < 6:
                        if mt == 3:
                            emit_qtile(p + 2, 0)
                        elif mt == 6:
                            emit_qtile(p + 2, 2)
                # post-window leftovers
                emit_av(0, 6, 0)
                emit_av(0, 6, 1)
                emit_av(0, 7, 0)
                emit_av(0, 7, 1)
                finish_head(p, 0, ups[0], usbs, rgs)
                emit_av(1, 5, 0)
                emit_av(1, 5, 1)
                emit_av(1, 6, 0)
                emit_av(1, 6, 1)
                emit_av(1, 7, 0)
                emit_av(1, 7, 1)
                finish_head(p, 1, ups[1], usbs, rgs)

            # ---- tail: recips(5), normalize(4), proj interleaved with
            # normalize(5) so PE never waits on the last chain
            yps = [None] * CT

            def proj_head(ct, pool, tag):
                yps[ct] = pool.tile([128, N], f32, tag=tag, name=f"yps{ct}")
                for c in range(2):
                    cs = slice(c * SC, (c + 1) * SC)
                    for f in range(CT - 2):
                        mm(yps[ct][:, cs],
                           wp_sb[f][:, ct * 128:(ct + 1) * 128],
                           ot[f][:, cs],
                           start=(f == 0), stop=False)

            def proj_mid(ct):
                f = CT - 2
                for c in range(2):
                    cs = slice(c * SC, (c + 1) * SC)
                    mm(yps[ct][:, cs],
                       wp_sb[f][:, ct * 128:(ct + 1) * 128],
                       ot[f][:, cs],
                       start=False, stop=False)

            def proj_last(ct):
                f = CT - 1
                ysb = pu.tile([128, N], f32, tag="ysb", bufs=2, name=f"ysb{ct}")
                for c in range(2):
                    cs = slice(c * SC, (c + 1) * SC)
                    mm(yps[ct][:, cs],
                       wp_sb[f][:, ct * 128:(ct + 1) * 128],
                       ot[f][:, cs],
                       start=False, stop=True)
                    if ct % 2 == 1:
                        nc.vector.tensor_copy(ysb[:, cs], yps[ct][:, cs])
                    else:
                        nc.scalar.copy(ysb[:, cs], yps[ct][:, cs])
                    eng = nc.sync if ct % 2 == 0 else nc.gpsimd
                    eng.dma_start(out=yt_d[ct * 128:(ct + 1) * 128, cs],
                                  in_=ysb[:, cs])

            # recips for pair 5 (its rg DMAs just issued above)
            emit_recips(5)

            emit_normalize(4)
            proj_head(0, psa, "st")
            proj_head(1, psa, "st")
            emit_normalize(5)
            proj_head(2, psu, "u")
            proj_head(3, psu, "u")
            for ct in range(4):
                proj_mid(ct)
            proj_last(0)
            proj_last(1)
            proj_head(4, psa, "st")
            proj_mid(4)
            proj_last(2)
            proj_head(5, psa, "st")
            proj_mid(5)
            proj_last(3)
            proj_last(4)
            proj_last(5)

            loop_stack.close()

    nc.compile()
    return nc


def _make_runner(nc):
    """Build the 8-core sharded jitted executable once (cached across calls)."""
    import jax
    import concourse.mybir as mybir
    from concourse import bass2jax
    from jax.experimental.shard_map import shard_map
    from jax.sharding import Mesh, PartitionSpec

    bass2jax.install_neuronx_cc_hook()
    partition_name = nc.partition_id_tensor.name if nc.partition_id_tensor else None
    in_names, out_names, out_avals, zero_outs = [], [], [], []
    for alloc in nc.m.functions[0].allocations:
        if not isinstance(alloc, mybir.MemoryLocationSet):
            continue
        name = alloc.memorylocations[0].name
        if alloc.kind == "ExternalInput":
            if name != partition_name:
                in_names.append(name)
        elif alloc.kind == "ExternalOutput":
            shape = tuple(alloc.tensor_shape)
            dtype = mybir.dt.np(alloc.dtype)
            out_names.append(name)
            out_avals.append(jax.core.ShapedArray(shape, dtype))
            zero_outs.append(np.zeros((B * shape[0], *shape[1:]), dtype))
    all_in_names = list(in_names) + list(out_names)
    if partition_name is not None:
        all_in_names.append(partition_name)

    def _body(*args):
        operands = list(args)
        if partition_name is not None:
            operands.append(bass2jax.partition_id_tensor())
        outs = bass2jax._bass_exec_p.bind(
            *operands,
            out_avals=tuple(out_avals),
            in_names=tuple(all_in_names),
            out_names=tuple(out_names),
            lowering_input_output_aliases=(),
            sim_require_finite=True,
            sim_require_nnan=True,
            nc=nc,
        )
        return tuple(outs)

    devices = jax.devices()[:B]
    mesh = Mesh(np.asarray(devices), ("core",))
    n_io = len(in_names) + len(out_avals)
    fn = jax.jit(shard_map(_body, mesh=mesh,
                           in_specs=(PartitionSpec("core"),) * n_io,
                           out_specs=(PartitionSpec("core"),) * len(out_avals),
                           check_rep=False))
    return fn, in_names, out_names, zero_outs


def kernel(x, qkv_w, proj_w, proj_b):
    global LAST_RESULT
    _ensure_path()
    import ml_dtypes

    bf16 = ml_dtypes.bfloat16
    x = np.asarray(x, dtype=np.float32)
    qkv_w = np.asarray(qkv_w, dtype=np.float32)
    proj_w = np.asarray(proj_w, dtype=np.float32)
    proj_b = np.asarray(proj_b, dtype=np.float32)

    if "runner" not in _CACHE:
        _CACHE["nc"] = _build_nc()
        _CACHE["runner"] = _make_runner(_CACHE["nc"])
    fn, in_names, out_names, zero_outs = _CACHE["runner"]

    wqk = np.ascontiguousarray(qkv_w[:2 * C].T).astype(bf16)
    wv = np.ascontiguousarray(qkv_w[2 * C:].T).astype(bf16)
    wp = np.ascontiguousarray(proj_w.T).astype(bf16)
    per_core = {
        "xt": np.concatenate(
            [np.ascontiguousarray(x[b].T).astype(bf16) for b in range(B)], axis=0),
        "wqk": np.concatenate([wqk] * B, axis=0),
        "wv": np.concatenate([wv] * B, axis=0),
        "wp": np.concatenate([wp] * B, axis=0),
    }
    args = [per_core[nm] for nm in in_names] + list(zero_outs)
    outs = fn(*args)
    yt = np.asarray(outs[out_names.index("yt")]).reshape(B, C, N)

    y = np.empty((B, N, C), dtype=np.float32)
    for b in range(B):
        y[b] = yt[b].T
    y += proj_b[None, None, :]
    return y

